# revision 94
# baseline (speedup 1.0000x reference)
"""Trainium2 Bass kernel for nn_Attention_43190191129190.

Model (per batch element b of 8):
    y   = x + dwconv3x3(x) + conv_b          (depthwise residual positional conv)
    qkv = y @ qkv_w.T ; split into q, k, v   (8 heads, dim 32)
    out = softmax(q k^T / sqrt(32)) v
    out = out @ out_w.T + out_b

Sharding: pure data-parallel, one batch element per NeuronCore (8 cores).

Per-core design (v2 — ScalarE-exp-bound schedule, 98.8us vs 130.1us v1):

  The 64 exp activations ([128,1024] each, one per (head, m-chunk)) are the
  irreducible ScalarE stream (64 x 1038ns = 66.4us); everything else is
  arranged to hide under it.  Final shape: exp0 at ~19.8us (PE-serial
  transposes+conv+qk ahead of it), a gapless exp stream, ~12.4us tail.

  1. x arrives as bf16 (host-cast; the extra 0.2% rounding on the residual
     is invisible next to the bf16 y^T cast) -> PE transposes (bf16
     identity) -> padded x^T; depthwise conv as 9 diagonal bf16 matmuls per
     128-channel tile (center tap +1.0 = residual), diagonals expanded
     on-chip from a 9KB vector (DVE/Pool) because DMA transfers serialize.
     conv bias is folded into the psum evacuation (tensor_scalar add) which
     also produces bf16 y^T.
  2. q^T/k^T in f32r (bf16 logits would double the error), from bf16
     y^T x bf16 qkv_w^T; v in bf16 with a per-head ones column ([v_h|1]).
     S^T(m0) is gated only on k's first m-chunk + q (split-engine evacs);
     the bulk of k^T evacuates after S^T(m0) is emitted.
  3. Per head pair, per m-chunk: S^T via K=32 f32r matmuls (2 heads in
     different 32-row PE groups via tile_position); exp on ScalarE straight
     from PSUM (scale folded; no max subtraction), output bf16.
  4. PV with exp(S^T) as the *stationary* operand (the cost model charges
     only output columns; the old moving-exp(S) formulation cost 4x more):
     out[n,(d|1)] per (head, n-chunk) accumulates over the 8 m-chunks as
     one consecutive burst of 8 matmuls (33-col output; bursts must be
     consecutive per psum bank - CoreSim's 2KB pending-zero granularity).
     A pair's 16 bursts run in the next pair's m-steps 0-3; normalization
     (m-step 4) is a per-partition tensor_scalar multiply split DVE/Pool
     (the denominator lands ON the partition that needs it - no broadcast);
     transposes back to attn^T run at m-step 6 (bf16 identity, output
     partition group 32*(h%4) via column tile position).
  5. Pairs are ordered (1,3),(0,2),(4,5),(6,7) so each hc1 pair's attn^T
     rows are contiguous (single row evacuation).  Out-projection:
     stationary attn^T chunks x moving out_w^T + K=1 ones-row bias matmul,
     two token chunks per psum tile/DMA, output DMAs split across queues.

  PSUM budget: 2 x st[128,1024] (S^T double-buffer, 4 banks) + one
  [128,2048] carve-out (conv accumulator, then qk-hc0/PV accumulators in
  banks 0-1, bf16 attn^T transpose target in bank 2, tail PV in bank 3).

  Scheduling facts this relies on (TimelineSim cost model): matmul cost =
  output free-dim size only (K, M, weight loads are free); fp32r needs
  >=256 output cols for full rate, bf16 is always full rate; dependency
  tracking is tile-granular (any read of a tile waits all earlier-emitted
  writes to it); gpsimd DMAs burn ~1us of Pool engine (SWDGE); DMA
  completion semaphores cost ~900ns; Pool cannot read PSUM; f32r matmul
  operands must be declared f32r, not bitcast from f32 (HW compile fails).
"""

import os

import numpy as np

import concourse.bass as bass
import concourse.tile as tile
from concourse import bacc, mybir
from concourse.bass_utils import run_bass_kernel_spmd

F32 = mybir.dt.float32
F32R = mybir.dt.float32r
BF16 = mybir.dt.bfloat16
AF = mybir.ActivationFunctionType
MUL = mybir.AluOpType.mult
ADD = mybir.AluOpType.add

B, N, C = 8, 1024, 256
HEADS, DH = 8, 32
SCALE = DH ** -0.5
PAD = 34  # 32x32 spatial grid with 1-px halo

TAPS = [(ky, kx) for ky in range(3) for kx in range(3)]
# pairs 0,1 complete heads 0-3 (attn^T chunk 0); pairs 2,3 complete 4-7.
# Each pair's heads differ in h%4 (distinct PE row groups for S^T); the
# hc1 pairs are chosen so each pair's attn^T rows are CONTIGUOUS
# (rows 0:64 / 64:128), making the tail row evacuation a single copy.
PAIRS = [(1, 3), (0, 2), (4, 5), (6, 7)]


def build_nc(debug_dump=False):
    nc = bacc.Bacc("TRN2", target_bir_lowering=False, debug=False, num_devices=8)

    x_d = nc.dram_tensor("x_bf", (N, C), BF16, kind="ExternalInput").ap()
    qkvwT_d = nc.dram_tensor("qkv_wT_bf", (C, 3 * C), BF16, kind="ExternalInput").ap()
    outwT_d = nc.dram_tensor("out_wT_bf", (C, C), BF16, kind="ExternalInput").ap()
    diagv_d = nc.dram_tensor("conv_diagv", (128, 18), F32, kind="ExternalInput").ap()
    convb_d = nc.dram_tensor("conv_b_r", (128, 2), F32, kind="ExternalInput").ap()
    outb_d = nc.dram_tensor("out_b_r", (1, C), BF16, kind="ExternalInput").ap()
    idb_d = nc.dram_tensor("id128b", (128, 128), BF16, kind="ExternalInput").ap()
    out_d = nc.dram_tensor("out", (N, C), F32, kind="ExternalOutput").ap()
    dbg = {}
    if debug_dump:
        for name, shape, dt in (
            ("d_yT", (128, 2, N), BF16), ("d_qT", (128, 2, N), F32),
            ("d_kT", (128, 2, N), F32), ("d_v", (128, 8, 8 * 33), BF16),
            ("d_attnT", (128, 2, N), BF16),
        ):
            dbg[name] = nc.dram_tensor(name, shape, dt, kind="ExternalOutput").ap()

    with tile.TileContext(nc) as tc:
        with (
            tc.tile_pool(name="const", bufs=1) as const,
            tc.tile_pool(name="xin", bufs=1) as xin_p,
            tc.tile_pool(name="big", bufs=1) as big,
            tc.tile_pool(name="pT", bufs=36) as ppool,
            tc.tile_pool(name="attnN", bufs=2) as an_p,
            tc.tile_pool(name="pvsb", bufs=2) as pvs_p,
            tc.tile_pool(name="rden", bufs=2) as rd_p,
            tc.tile_pool(name="outs", bufs=4) as outs_p,
            tc.tile_pool(name="pst", bufs=2, space="PSUM") as pst,
            tc.tile_pool(name="misc", bufs=1, space="PSUM") as miscp,
        ):
            # ---- DMAs: id + x tiles first (startup critical path), weights
            # after; x loads spread over three DGE queues
            # DMA transfers serialize on the DMA-engine resource, so the big
            # conv-diag matrices are NOT shipped: only their 9KB diagonal,
            # expanded on-chip (DVE for ct0, Pool for ct1).  gpsimd DMAs cost
            # ~1us of Pool ENGINE time each (SWDGE runs on the Q7s), so only
            # 2 x tiles go there.
            # x-pair0 leads the sync queue (the PE's first dependency);
            # idb rides SWDGE on gpsimd so it doesn't push x back
            idb_sb = const.tile([128, 128], BF16, tag="idb")
            nc.gpsimd.dma_start(idb_sb, idb_d)
            diagv_sb = const.tile([128, 18], F32, tag="diagv")
            nc.scalar.dma_start(diagv_sb, diagv_d)
            # x in 4 double-tile transfers (amortizes the ~900ns DMA
            # completion semaphores), alternating sync/scalar/gpsimd
            xins = []
            _dma_engines = [nc.sync, nc.scalar, nc.gpsimd]
            _xq = [nc.sync, nc.scalar, nc.gpsimd, nc.sync]
            for jp in range(4):
                xin = xin_p.tile([128, 2, C], BF16, tag=f"xin{jp}", name=f"xin{jp}")
                _xq[jp].dma_start(
                    xin,
                    x_d[jp * 256:(jp + 1) * 256, :].rearrange(
                        "(c p) f -> p c f", p=128),
                )
                xins.append(xin)
            qkvwT_sb = const.tile([128, 2, 3 * C], BF16, tag="qkvwT")
            nc.scalar.dma_start(qkvwT_sb, qkvwT_d.rearrange("(kc p) f -> p kc f", p=128))
            convb_sb = const.tile([128, 2], F32, tag="convb")
            nc.sync.dma_start(convb_sb, convb_d)
            outwT_sb = const.tile([128, 2, C], BF16, tag="outwT")
            nc.sync.dma_start(outwT_sb, outwT_d.rearrange("(kc p) f -> p kc f", p=128))
            outb_sb = const.tile([1, C], BF16, tag="outb")
            nc.sync.dma_start(outb_sb, outb_d)
            zerob_sb = const.tile([128, 1], F32, tag="zerob")
            nc.vector.memset(zerob_sb, 0.0)
            # ones strip (bf16): K=1 stationary for the bias matmul + dummy
            # PE warm-up fodder (DVE memset: Pool is busy dispatching DMAs)
            onesb_sb = const.tile([1, 512], BF16, tag="onesb")
            nc.vector.memset(onesb_sb, 1.0)
            # dummy exp: hoists the ~1.3us Exp ACT table load into the idle
            # startup window
            warm_sb = const.tile([1, 1], F32, tag="warm")
            nc.scalar.activation(
                warm_sb, zerob_sb[0:1, 0:1], AF.Exp,
                bias=zerob_sb[0:1], scale=1.0,
            )

            # ---- persistent activations ----
            xpadT = big.tile([128, 2, PAD * PAD], BF16, tag="xpadT")
            xpv = xpadT.rearrange("p ct (h w) -> p ct h w", h=PAD)
            nc.vector.memset(xpv[:, :, 0, :], 0.0)
            nc.vector.memset(xpv[:, :, PAD - 1, :], 0.0)
            nc.vector.memset(xpv[:, :, :, 0], 0.0)
            nc.vector.memset(xpv[:, :, :, PAD - 1], 0.0)
            yT = big.tile([128, 2, N], BF16, tag="yT")
            qT0 = big.tile([128, 2, 512], F32R, tag="qT0")
            qT1 = big.tile([128, 2, 512], F32R, tag="qT1")
            kT = big.tile([128, 2, N], F32R, tag="kT")
            # v: per m-chunk, per head: [v_h | 1] (33 bf16 cols); ones from a
            # whole-tile memset, v cols overwritten by the evacuations
            vsb = big.tile([128, 8, HEADS, 33], BF16, tag="v")
            attnT_sb = big.tile([128, 2, N], BF16, tag="attnT")

            # PE warm-up: cheap dummy matmuls during the x-DMA wait so the
            # p-state ramp starts before the first transpose
            wps = pst.tile([128, 1024], F32, tag="ps", name="wps")
            for i in range(4):
                nc.tensor.matmul(
                    wps[:, 0:512], lhsT=onesb_sb[0:1, 0:128], rhs=onesb_sb,
                    start=True, stop=True, skip_group_check=True,
                )

            diag_sb = big.tile([128, 18, 128], BF16, tag="diag")

            def emit_diag():
                # expand the conv diagonals: diag_t = id * diagv[:, t] (per-
                # partition scalar); emitted after the first transposes so
                # the x evacuations lead the DVE queue (diagv's DMA
                # completion sem lands ~3.7us anyway); ct1 taps on Pool
                for t in range(6):
                    nc.vector.tensor_scalar(
                        diag_sb[:, t, :], idb_sb, diagv_sb[:, t:t + 1], None, MUL)
                for t in range(6, 18):
                    nc.gpsimd.tensor_scalar(
                        diag_sb[:, t, :], idb_sb, diagv_sb[:, t:t + 1], None, MUL)

            # pre-exp psum evacuations alternate DVE / (still idle) ScalarE
            _cp = [0]

            def copy_alt(dst, src_ap):
                _cp[0] += 1
                if _cp[0] % 2:
                    nc.vector.tensor_copy(dst, src_ap)
                else:
                    nc.scalar.copy(dst, src_ap)

            # ---- transpose x into padded x^T (f32r: 1.5 c/row vs fp32's
            # 2.0; evacs DVE-only so ScalarE stays on the diag expansion) ----
            def emit_transpose(nt):
                tp = pst.tile([128, 1024], F32, tag="ps", name="tp").bitcast(BF16)
                for ct in range(2):
                    nc.tensor.transpose(
                        tp[:, 512 * ct: 512 * ct + 128],
                        xins[nt // 2][:, nt % 2, 128 * ct: 128 * (ct + 1)],
                        idb_sb,
                    )
                    dst = xpadT[:, ct, :].rearrange("p (h w) -> p h w", h=PAD)[
                        :, 1 + 4 * nt: 5 + 4 * nt, 1:33
                    ]
                    copy_alt(
                        dst,
                        tp[:, 512 * ct: 512 * ct + 128].rearrange(
                            "p (a b) -> p a b", a=4
                        ),
                    )

            # conv accumulator in the misc psum slot ([128,2048], 4 banks)
            cacc = miscp.tile([128, 2048], F32, tag="misc", name="cacc")

            def emit_conv_half(ct, j, cps):
                view = xpadT[:, ct, :].rearrange("p (h w) -> p h w", h=PAD)
                for t, (ky, kx) in enumerate(TAPS):
                    nc.tensor.matmul(
                        cps,
                        lhsT=diag_sb[:, ct * 9 + t, :],
                        rhs=view[:, ky + 16 * j: ky + 16 * j + 16, kx: kx + 32],
                        start=(t == 0),
                        stop=(t == 8),
                    )

            def emit_yevac(ct, j, cps, eng):
                # psum -> bf16 y^T with the conv bias folded in (Pool cannot
                # read PSUM on TRN2, so only ScalarE/DVE evacuate psum)
                eng_map = {
                    "s": lambda o, i, s: nc.scalar.activation(
                        o, i, AF.Identity, bias=s, scale=1.0),
                    "v": lambda o, i, s: nc.vector.tensor_scalar(
                        o, i, s, None, ADD),
                }
                eng_map[eng](yT[:, ct, j * 512:(j + 1) * 512], cps,
                             convb_sb[:, ct:ct + 1])

            for nt in range(3):
                emit_transpose(nt)
            emit_diag()
            for nt in range(3, 5):
                emit_transpose(nt)
            emit_conv_half(0, 0, cacc[:, 0:512])
            emit_conv_half(1, 0, cacc[:, 1024:1536])
            for nt in range(5, 8):
                emit_transpose(nt)
            # conv j1 in two separate pst tiles so each ct's evacuation can
            # start the moment its own 9 taps finish (tile-granular deps)
            c1a = pst.tile([128, 1024], F32, tag="ps", name="c1a")
            emit_conv_half(0, 1, c1a[:, 0:512])
            c1b = pst.tile([128, 1024], F32, tag="ps", name="c1b")
            emit_yevac(0, 1, c1a[:, 0:512], "s")
            emit_conv_half(1, 1, c1b[:, 0:512])
            emit_yevac(0, 0, cacc[:, 0:512], "s")
            emit_yevac(1, 0, cacc[:, 1024:1536], "v")
            emit_yevac(1, 1, c1b[:, 0:512], "v")

            # ---- q^T / k^T feature tiles (f32r from psum; hc0 tiles now,
            # hc1 deferred into pair 0's m-loop) ----
            def emit_qk(ft, qps, evac=None, kcs=(0, 1), do_evac=True):
                # qps: caller-provided [128, 1024] psum region (the hc1 tiles
                # deferred into pair 0 use the then-idle misc banks so they
                # don't rotate the S^T double-buffer pool); kcs lets pair 0
                # spread the contraction over two m-steps so the S^T stream
                # never stalls behind a full 8-matmul block
                dc = ft if ft < 2 else ft - 2
                fofs = 0 if ft < 2 else 256
                for kc in kcs:
                    for j in range(2):
                        nc.tensor.matmul(
                            qps[:, j * 512:(j + 1) * 512],
                            lhsT=qkvwT_sb[:, kc, fofs + dc * 128: fofs + (dc + 1) * 128],
                            rhs=yT[:, kc, j * 512:(j + 1) * 512],
                            start=(kc == 0),
                            stop=(kc == 1),
                        )
                if do_evac:
                    ev = evac or (copy_alt if evac is None else evac)
                    if ft < 2:
                        # q lives in two j-half tiles so S^T's j0 matmuls
                        # never wait on the j1 evacuation (tile-granular deps)
                        ev(qT0[:, dc, :], qps[:, 0:512])
                        ev(qT1[:, dc, :], qps[:, 512:1024])
                    else:
                        ev(kT[:, dc, :], qps)

            def emit_v(nt, vps):
                for kc in range(2):
                    nc.tensor.matmul(
                        vps[:, 0:256],
                        lhsT=yT[:, kc, nt * 128:(nt + 1) * 128],
                        rhs=qkvwT_sb[:, kc, 512:768],
                        start=(kc == 0),
                        stop=(kc == 1),
                    )
                vv = vps[:, 0:256].rearrange("p (hh c) -> p hh c", c=32)
                nc.vector.tensor_copy(vsb[:, nt, :, 0:32], vv)

            # v's ones columns (Pool is free by now; needed from pair-0 m2)
            nc.gpsimd.memset(vsb, 1.0)

            # ---- attention ----
            # misc psum carve-out for the pair phase: pv head-slot ih lives
            # in bank ih (groups are 33 cols and must not cross a bank);
            # attn^T transpose target = bank 2 viewed as bf16
            pvt = miscp.tile([128, 2048], F32, tag="misc", name="pvt")
            pv = pvt[:, 0:1024]
            attnT_ps = pvt[:, 1024:1536].bitcast(BF16)

            # hc0 q/k tiles accumulate in the misc banks (keeping both pst
            # slots free for S^T(m0)); only the minimal evacuations gate the
            # first S^T: k's m0 chunk (ScalarE) + q in two engine-parallel
            # halves.  The rest of k is evacuated inside pair-0 m0, after
            # S^T(m0) is emitted, so nothing waits on it.
            emit_qk(2, pvt[:, 0:1024], do_evac=False)
            nc.scalar.copy(kT[:, 0, 0:128], pvt[:, 0:128])
            qps_q = pst.tile([128, 1024], F32, tag="ps", name="qpsq")
            emit_qk(0, qps_q, do_evac=False)
            nc.vector.tensor_copy(qT0[:, 0, :], qps_q[:, 0:512])
            nc.scalar.copy(qT1[:, 0, :], qps_q[:, 512:1024])

            pT_tiles = {}  # (pair, ih, m) -> tile

            def emit_pv_burst(ip, ih, nch, h, bank=None):
                # one (head, n-chunk) group: 8 consecutive matmuls, exp(S^T)
                # chunks stationary, [v_h|1] moving, accumulated over m
                base = 512 * (ih if bank is None else bank)
                for m in range(8):
                    nc.tensor.matmul(
                        pvt[:, base + 33 * nch: base + 33 * nch + 33],
                        lhsT=pT_tiles[(ip, ih, m)][:, nch * 128:(nch + 1) * 128],
                        rhs=vsb[:, m, h, :],
                        start=(m == 0),
                        stop=(m == 7),
                    )

            def emit_norms_ih(ip, ih, attnN, evac=None, bank=None, pvsb=None):
                # evacuate one head's pv bank, reciprocal of the denominators,
                # then the per-partition normalize (n is the partition dim, so
                # no broadcast is needed), alternating DVE/Pool
                base = 512 * (ih if bank is None else bank)
                if pvsb is None:
                    pvsb = pvs_p.tile([128, 264], F32, tag="pvsb")
                    (evac or nc.vector.tensor_copy)(pvsb, pvt[:, base: base + 264])
                rden = rd_p.tile([128, 8], F32, tag="rden")
                nc.vector.reciprocal(
                    rden,
                    bass.AP(tensor=pvsb.tensor, offset=pvsb.offset + 32,
                            ap=[list(pvsb.ap[0]), [33, 8]]),
                )
                for nch in range(8):
                    eng = nc.vector if (nch + ih) % 2 else nc.gpsimd
                    eng.tensor_scalar(
                        attnN[:, ih, nch, :],
                        pvsb[:, 33 * nch: 33 * nch + 32],
                        rden[:, nch: nch + 1],
                        None, MUL,
                    )

            def emit_norms(ip):
                # both heads at once: one strided pv evacuation, one
                # reciprocal, 16 normalizes alternating DVE/Pool
                attnN = an_p.tile([128, 2, 8, 32], BF16, tag="attnN")
                pvsb = pvs_p.tile([128, 2, 264], F32, tag="pvsb2", name="pvsb2")
                nc.vector.tensor_copy(
                    pvsb,
                    bass.AP(tensor=pv.tensor, offset=pv.offset,
                            ap=[list(pv.ap[0]), [512, 2], [1, 264]]),
                )
                rden = rd_p.tile([128, 2, 8], F32, tag="rden2", name="rden2")
                nc.vector.reciprocal(
                    rden,
                    bass.AP(tensor=pvsb.tensor, offset=pvsb.offset + 32,
                            ap=[list(pvsb.ap[0]), [264, 2], [33, 8]]),
                )
                for nch in range(8):
                    for ih in range(2):
                        eng = nc.vector if (nch + ih) % 2 else nc.gpsimd
                        eng.tensor_scalar(
                            attnN[:, ih, nch, :],
                            pvsb[:, ih, 33 * nch: 33 * nch + 32],
                            rden[:, ih, nch: nch + 1],
                            None, MUL,
                        )
                return attnN

            def emit_transposes_ih(ip, ih, attnN):
                h = PAIRS[ip][ih]
                a = 32 * (h % 4)
                for nch in range(8):
                    nc.tensor.transpose(
                        attnT_ps[a:a + 32, nch * 128:(nch + 1) * 128],
                        attnN[:, ih, nch, :],
                        idb_sb,
                        tile_position=(0, a),
                    )

            def emit_rowevac(ip, ih, eng):
                h = PAIRS[ip][ih]
                a = 32 * (h % 4)
                eng(attnT_sb[a:a + 32, h // 4, :], attnT_ps[a:a + 32, :])

            def emit_transposes(ip, attnN):
                # PE transposes into attn^T (column tile position 32*(h%4)),
                # then the pair's row evacuation (one copy when the two
                # heads' row groups are contiguous) so bank 2 frees each pair
                emit_transposes_ih(ip, 0, attnN)
                emit_transposes_ih(ip, 1, attnN)
                hA, hB = PAIRS[ip]
                a0, a1 = sorted((32 * (hA % 4), 32 * (hB % 4)))
                if a1 - a0 == 32:
                    nc.vector.tensor_copy(
                        attnT_sb[a0:a0 + 64, hA // 4, :], attnT_ps[a0:a0 + 64, :]
                    )
                else:
                    emit_rowevac(ip, 0, nc.vector.tensor_copy)
                    emit_rowevac(ip, 1, nc.vector.tensor_copy)

            attnN_t = {}

            def pair_extra(ip, m):
                # deferred work slotted into the m-steps: pair 0 absorbs the
                # hc1 q/k tiles + v (psum carved from the then-idle misc
                # banks); later pairs run the previous pair's PV bursts
                # (4 per step, done by m=3) and normalization (m=4)
                if ip == 0:
                    if m < 4:
                        if m == 0:
                            # deferred bulk of k's evacuation (S^T(m0) is
                            # already emitted, so only S^T(m1)+ wait on it)
                            nc.vector.tensor_copy(
                                kT[:, 0, 128:1024], pvt[:, 128:1024])
                        # q/k hc1 tiles, half a contraction per m-step
                        ft = 1 if m < 2 else 3
                        qps = pvt[:, 0:1024] if m < 2 else pvt[:, 1024:2048]
                        emit_qk(ft, qps, evac=nc.vector.tensor_copy,
                                kcs=(m % 2,), do_evac=(m % 2 == 1))
                    else:
                        for nt in (2 * m - 8, 2 * m - 7):
                            emit_v(nt, pvt[:, 256 * (nt % 4): 256 * (nt % 4) + 256])
                elif m < 4:
                    pp = ip - 1
                    hA, hB = PAIRS[pp]
                    for nch in (2 * m, 2 * m + 1):
                        for ih, h in ((0, hA), (1, hB)):
                            emit_pv_burst(pp, ih, nch, h)
                elif m == 4:
                    attnN_t[ip - 1] = emit_norms(ip - 1)
                elif m == 6:
                    # transposes run during this pair's exp stream; emitting
                    # them here (not after the loop) frees psum bank 2 well
                    # before the tail's PV bursts need it
                    emit_transposes(ip - 1, attnN_t[ip - 1])

            for ip, (hA, hB) in enumerate(PAIRS):
                for m in range(8):
                    # head-major: head A's exp is emitted right after its two
                    # S^T matmuls so its psum slot turns over one matmul
                    # earlier (shrinks the pair-entry transient)
                    for ih, h in ((0, hA), (1, hB)):
                        st = pst.tile([128, 1024], F32, tag="ps")
                        a = 32 * (h % 4)
                        hc = h // 4
                        pT = ppool.tile([128, 1024], BF16, tag="pT")
                        if ip == 0 and m == 0:
                            # the very first exps run as halves so the
                            # ScalarE stream starts right after S^T-j0
                            # (which only waits on the qT0 evacuation)
                            for j in range(2):
                                nc.tensor.matmul(
                                    st[:, j * 512:(j + 1) * 512],
                                    lhsT=kT[a:a + 32, hc, m * 128:(m + 1) * 128],
                                    rhs=(qT0 if j == 0 else qT1)[a:a + 32, hc, :],
                                    start=True,
                                    stop=True,
                                    tile_position=(a, 0),
                                )
                                nc.scalar.activation(
                                    pT[:, j * 512:(j + 1) * 512],
                                    st[:, j * 512:(j + 1) * 512],
                                    AF.Exp, bias=zerob_sb, scale=SCALE)
                        else:
                            for j in range(2):
                                nc.tensor.matmul(
                                    st[:, j * 512:(j + 1) * 512],
                                    lhsT=kT[a:a + 32, hc, m * 128:(m + 1) * 128],
                                    rhs=(qT0 if j == 0 else qT1)[a:a + 32, hc, :],
                                    start=True,
                                    stop=True,
                                    tile_position=(a, 0),
                                )
                            nc.scalar.activation(pT, st, AF.Exp, bias=zerob_sb, scale=SCALE)
                        pT_tiles[(ip, ih, m)] = pT
                    pair_extra(ip, m)

            def emit_proj(sc):
                # two token chunks per psum tile / evac / DMA: halves the
                # per-queue HWDGE setups and the evac count in the tail
                ops = pst.tile([128, 1024], F32, tag="ps")
                for cc in range(2):
                    nch = 2 * sc + cc
                    for hc in range(2):
                        nc.tensor.matmul(
                            ops[:, 512 * cc: 512 * cc + 256],
                            lhsT=attnT_sb[:, hc, nch * 128:(nch + 1) * 128],
                            rhs=outwT_sb[:, hc, :],
                            start=(hc == 0),
                            stop=False,
                        )
                    nc.tensor.matmul(
                        ops[:, 512 * cc: 512 * cc + 256],
                        lhsT=onesb_sb[0:1, 0:128],
                        rhs=outb_sb,
                        start=False,
                        stop=True,
                    )
                osb = outs_p.tile([128, 2, C], F32, tag="o")
                src = bass.AP(tensor=ops.tensor, offset=ops.offset,
                              ap=[list(ops.ap[0]), [512, 2], [1, 256]])
                if sc % 2:
                    nc.vector.tensor_copy(osb, src)
                else:
                    nc.scalar.copy(osb, src)
                # sc1 via gpsimd (Pool is idle in the tail) so the scalar
                # queue's 667ns DMA dispatch never lands between ScalarE's
                # osb copies; the last chunk stays on the fast HWDGE path
                eng = [nc.sync, nc.gpsimd, nc.sync, nc.scalar][sc]
                eng.dma_start(
                    out_d[sc * 256:(sc + 1) * 256, :].rearrange(
                        "(cc p) f -> p cc f", p=128),
                    osb)

            # ---- tail: last pair's PV, normalize, then a per-token-chunk
            # pipeline: transpose -> 32x128 row evacs (ScalarE+DVE) ->
            # project -> store, so chunk k's DMA overlaps chunk k+1's math
            # the last pair's PV lands in banks 3 and 2 (free since pair 2's
            # norms/evac) so both head bursts run back-to-back with no WAR
            # on the pair-2 pv region; both evacuations then run in parallel
            # (ScalarE + DVE), then norms/transposes pipeline per head
            hA3, hB3 = PAIRS[3]
            attnN3v = an_p.tile([128, 2, 8, 32], BF16, tag="attnN")
            for nch in range(8):
                emit_pv_burst(3, 0, nch, hA3, bank=3)
            # head A's pv evacuated on ScalarE while head B's bursts run
            pvsb0 = pvs_p.tile([128, 264], F32, tag="pvsb", name="pvsb30")
            nc.scalar.copy(pvsb0, pvt[:, 1536:1800])
            for nch in range(8):
                emit_pv_burst(3, 1, nch, hB3, bank=2)
            pvsb1 = pvs_p.tile([128, 264], F32, tag="pvsb", name="pvsb31")
            nc.vector.tensor_copy(pvsb1, pvt[:, 1024:1288])
            emit_norms_ih(3, 0, attnN3v, pvsb=pvsb0)
            emit_transposes_ih(3, 0, attnN3v)
            emit_norms_ih(3, 1, attnN3v, pvsb=pvsb1)
            emit_transposes_ih(3, 1, attnN3v)
            # heads 6,7 -> rows 64:128: one contiguous row evacuation
            nc.vector.tensor_copy(attnT_sb[64:128, 1, :], attnT_ps[64:128, :])
            for sc in range(4):
                emit_proj(sc)

            if debug_dump:
                nc.sync.dma_start(dbg["d_yT"], yT)
                nc.sync.dma_start(dbg["d_kT"], kT.bitcast(F32))
                nc.sync.dma_start(dbg["d_v"], vsb.rearrange("p m h c -> p m (h c)"))
                nc.sync.dma_start(dbg["d_attnT"], attnT_sb)

    nc.compile()
    return nc


_NC = None
LAST_RESULTS = None


def _host_prep(conv_w, conv_b, qkv_w, out_w, out_b):
    import ml_dtypes

    conv_w = np.asarray(conv_w, np.float32).reshape(C, 3, 3)
    diagv = np.zeros((128, 18), np.float32)
    for ct in range(2):
        for t, (ky, kx) in enumerate(TAPS):
            d = conv_w[128 * ct: 128 * (ct + 1), ky, kx].copy()
            if (ky, kx) == (1, 1):
                d += 1.0  # residual connection folded into the center tap
            diagv[:, ct * 9 + t] = d
    bf = ml_dtypes.bfloat16
    return {
        "qkv_wT_bf": np.ascontiguousarray(
            np.asarray(qkv_w, np.float32).T).astype(bf),
        "out_wT_bf": np.ascontiguousarray(
            np.asarray(out_w, np.float32).T).astype(bf),
        "conv_diagv": diagv,
        "conv_b_r": np.ascontiguousarray(
            np.asarray(conv_b, np.float32).reshape(2, 128).T),
        "out_b_r": np.asarray(out_b, np.float32).reshape(1, C).astype(bf),
        "id128b": np.eye(128, dtype=np.float32).astype(bf),
    }


def kernel(x, conv_w, conv_b, qkv_w, out_w, out_b):
    global _NC, LAST_RESULTS
    if _NC is None:
        _NC = build_nc()
    import ml_dtypes

    x = np.asarray(x, np.float32).astype(ml_dtypes.bfloat16)
    shared = _host_prep(conv_w, conv_b, qkv_w, out_w, out_b)
    in_maps = [{**shared, "x_bf": np.ascontiguousarray(x[b])} for b in range(B)]
    trace = bool(int(os.environ.get("KERNEL_TRACE", "0")))
    try:
        res = run_bass_kernel_spmd(_NC, in_maps, core_ids=list(range(B)), trace=trace)
    except Exception:
        if not trace:
            raise
        res = run_bass_kernel_spmd(_NC, in_maps, core_ids=list(range(B)), trace=False)
    LAST_RESULTS = res
    return np.stack([res.results[b]["out"] for b in range(B)], axis=0)
